# revision 1
# baseline (speedup 1.0000x reference)
"""CGCNNConv forward on 8 Trainium2 NeuronCores (Bass/Tile).

Math (per atom i, neighbor slot m):
  combined = [atom[i] | atom[nbr[i,m]] | bond[i,m]]          # 640
  z        = combined @ fc_w.T + fc_b                        # 512
  z        = LN(z) * ln1_g + ln1_b
  out[i]   = atom[i] + LN( mean_m sigmoid(z[:256]) * softplus(z[256:]) ) * ln2_g + ln2_b

Sharding: atoms split across 8 cores (padded 30000 -> 30720 = 8*3840).
atom_feats is replicated to every core's HBM so the neighbor gather is a
local indirect DMA.

Device layout per core (supertile = 128 atoms = 1536 (atom,m) rows = 12
row-tiles of 128 rows):
  - atom contribution is computed once per atom (atomT stationary, W1^T
    moving) with fc_b folded in, then expanded to rows with a one-hot
    matmul (E).
  - neighbor rows are gathered [128,256] row-major and PE-transposed so
    the feature dim lands on partitions for the main matmuls.
  - bond features arrive pre-transposed from the host.
  - z accumulates in PSUM [128 rows, 512]; LN1 stats via bn_stats; the
    per-row (scale,bias) is fused into the ACT sigmoid/softplus reads.
  - mean over m via a one-hot 1/12 matmul (G) accumulating [128 atoms,256]
    across the 12 row-tiles of a supertile; LN2 + residual epilogue.
All matmuls run in float32r (full-rate fp32 mode on TRN2 PE).
"""

import os
import sys

import numpy as np

sys.path.insert(0, "/opt/trn_rl_repo")
os.environ.setdefault("NEURON_COMPILE_CACHE_URL", "/root/neff_cache")

N, M, A, B = 30000, 12, 256, 128
NCORES = 8
NS = 3840                    # atoms per core (padded)
NPAD = NS * NCORES           # 30720
SA = 128                     # atoms per supertile
NSUPER = NS // SA            # 30
RT = 12                      # row-tiles per supertile
ST_ROWS = SA * M             # 1536
ROWS = NS * M                # 46080
K2A = 2 * A                  # 512
KIN = 2 * A + B              # 640
NCHUNK = KIN // 128          # 5
LN_EPS = 1e-5

_CACHE = {}
_NEFF_CACHE_DIR = os.environ.get("NEFF_DISK_CACHE", "/root/neff_cache")
_cache_installed = False


def _install_neff_cache():
    """Cache compiled NEFFs on disk keyed by BIR hash (compile takes ~40min)."""
    global _cache_installed
    if _cache_installed:
        return
    _cache_installed = True
    import hashlib
    import shutil

    from concourse import bass2jax, bass_utils

    orig = bass_utils.compile_bir_kernel

    def cached(bir_json, tmpdir, neff_name="file.neff"):
        try:
            os.makedirs(_NEFF_CACHE_DIR, exist_ok=True)
            h = hashlib.sha256(bir_json).hexdigest()[:32]
            cpath = os.path.join(_NEFF_CACHE_DIR, h + ".neff")
            if os.path.exists(cpath):
                dst = os.path.join(tmpdir, neff_name)
                shutil.copy(cpath, dst)
                return dst
        except Exception:
            cpath = None
        out = orig(bir_json, tmpdir, neff_name)
        if cpath is not None:
            try:
                shutil.copy(out, cpath)
            except Exception:
                pass
        return out

    bass_utils.compile_bir_kernel = cached
    bass2jax.compile_bir_kernel = cached


def _expand_maps():
    """E: [atom a, (j,r)] one-hot; G: [row r, (j,a)] one-hot / 12."""
    emat = np.zeros((SA, RT * 128), dtype=np.float32)
    gmat = np.zeros((128, RT * 128), dtype=np.float32)
    for j in range(RT):
        for r in range(128):
            a = (128 * j + r) // M
            emat[a, j * 128 + r] = 1.0
            gmat[r, j * 128 + a] = 1.0 / M
    return emat, gmat


def _build(general_ln1, general_ln2):
    import concourse.bass as bass
    import concourse.tile as tile
    from concourse import bacc, mybir

    f32 = mybir.dt.float32
    f32r = mybir.dt.float32r
    i32 = mybir.dt.int32
    AF = mybir.ActivationFunctionType

    nc = bacc.Bacc("TRN2", target_bir_lowering=False, debug=False,
                   num_devices=NCORES)

    d_atom_full = nc.dram_tensor("atom_full", [N, A], f32r, kind="ExternalInput")
    d_atomT = nc.dram_tensor("atomT2", [128, 2 * NS], f32r, kind="ExternalInput")
    d_atom_rows = nc.dram_tensor("atom_rows", [NS, A], f32, kind="ExternalInput")
    d_bondT = nc.dram_tensor("bondT", [B, ROWS], f32r, kind="ExternalInput")
    d_idx = nc.dram_tensor("nbr_idx", [ROWS], i32, kind="ExternalInput")
    d_wt = nc.dram_tensor("wt", [128, NCHUNK * K2A], f32r, kind="ExternalInput")
    d_fcb = nc.dram_tensor("fcb_rep", [128, K2A], f32, kind="ExternalInput")
    d_emat = nc.dram_tensor("emat", [SA, RT * 128], f32r, kind="ExternalInput")
    d_gmat = nc.dram_tensor("gmat", [128, RT * 128], f32r, kind="ExternalInput")
    d_ident = nc.dram_tensor("ident", [128, 128], f32r, kind="ExternalInput")
    if general_ln1:
        d_g1 = nc.dram_tensor("ln1g_rep", [128, K2A], f32, kind="ExternalInput")
        d_b1 = nc.dram_tensor("ln1b_rep", [128, K2A], f32, kind="ExternalInput")
    if general_ln2:
        d_g2 = nc.dram_tensor("ln2g_rep", [128, A], f32, kind="ExternalInput")
        d_b2 = nc.dram_tensor("ln2b_rep", [128, A], f32, kind="ExternalInput")
    d_out = nc.dram_tensor("out", [NS, A], f32, kind="ExternalOutput")

    r = lambda ap: ap if ap.dtype == f32r else ap.bitcast(f32r)

    with tile.TileContext(nc) as tc:
        with (
            tc.tile_pool(name="const", bufs=1) as cpool,
            tc.tile_pool(name="io", bufs=2) as iopool,
            tc.tile_pool(name="work", bufs=3) as wpool,
            tc.tile_pool(name="stat", bufs=4) as spool,
            tc.tile_pool(name="zps", bufs=2, space="PSUM") as zpool,
            tc.tile_pool(name="tps", bufs=2, space="PSUM") as tpool,
            tc.tile_pool(name="aps", bufs=2, space="PSUM") as apool,
            tc.tile_pool(name="gps", bufs=2, space="PSUM") as gpool,
        ):
            # ---- resident constants ----
            wt = cpool.tile([128, NCHUNK * K2A], f32r, tag="wt")
            nc.sync.dma_start(wt[:], d_wt[:])
            atomT = cpool.tile([128, 2 * NS], f32r, tag="atomT")
            nc.sync.dma_start(atomT[:], d_atomT[:])
            fcb = cpool.tile([128, K2A], f32, tag="fcb")
            nc.sync.dma_start(fcb[:], d_fcb[:])
            emat = cpool.tile([SA, RT * 128], f32r, tag="emat")
            nc.sync.dma_start(emat[:], d_emat[:])
            gmat = cpool.tile([128, RT * 128], f32r, tag="gmat")
            nc.sync.dma_start(gmat[:], d_gmat[:])
            ident = cpool.tile([128, 128], f32r, tag="ident")
            nc.sync.dma_start(ident[:], d_ident[:])
            eps_t = cpool.tile([128, 1], f32, tag="eps")
            nc.gpsimd.memset(eps_t[:], LN_EPS)
            ones_t = cpool.tile([128, 1], f32, tag="ones")
            nc.gpsimd.memset(ones_t[:], 1.0)
            if general_ln1:
                g1 = cpool.tile([128, K2A], f32, tag="g1")
                nc.sync.dma_start(g1[:], d_g1[:])
                b1 = cpool.tile([128, K2A], f32, tag="b1")
                nc.sync.dma_start(b1[:], d_b1[:])
            if general_ln2:
                g2 = cpool.tile([128, A], f32, tag="g2")
                nc.sync.dma_start(g2[:], d_g2[:])
                b2 = cpool.tile([128, A], f32, tag="b2")
                nc.sync.dma_start(b2[:], d_b2[:])

            for s in range(NSUPER):
                row0 = s * ST_ROWS
                # indices for this supertile: [p, j] = flat[row0 + 128j + p]
                idx = iopool.tile([128, RT], i32, tag="idx")
                nc.sync.dma_start(
                    idx[:],
                    d_idx[row0:row0 + ST_ROWS].rearrange("(j p) -> p j", p=128),
                )
                # gathered neighbor rows [p, j*256:(j+1)*256]
                nbr_g = iopool.tile([128, RT * A], f32r, tag="nbr_g")
                for j in range(RT):
                    nc.gpsimd.indirect_dma_start(
                        out=nbr_g[:, j * A:(j + 1) * A],
                        out_offset=None,
                        in_=d_atom_full[:],
                        in_offset=bass.IndirectOffsetOnAxis(
                            ap=idx[:, j:j + 1], axis=0),
                    )
                # bond^T slice [128 feat, 1536 rows]
                bondT = iopool.tile([B, ST_ROWS], f32r, tag="bondT")
                nc.sync.dma_start(bondT[:], d_bondT[:, row0:row0 + ST_ROWS])
                # residual rows
                arows = iopool.tile([SA, A], f32, tag="arows")
                nc.sync.dma_start(arows[:], d_atom_rows[s * SA:(s + 1) * SA, :])

                # atom contribution for these 128 atoms: [128 atoms, 512]
                ap_ps = apool.tile([SA, K2A], f32, tag="ap_ps")
                for c in range(2):
                    nc.tensor.matmul(
                        out=ap_ps[:],
                        lhsT=r(atomT[:, c * NS + s * SA: c * NS + (s + 1) * SA]),
                        rhs=r(wt[:, c * K2A:(c + 1) * K2A]),
                        start=(c == 0), stop=(c == 1),
                    )
                ap_sb = wpool.tile([SA, K2A], f32r, tag="ap_sb")
                nc.vector.tensor_add(out=ap_sb[:], in0=ap_ps[:], in1=fcb[:])

                agg = gpool.tile([SA, A], f32, tag="agg")

                for j in range(RT):
                    # transpose gathered neighbors -> [feat, rows]
                    tp = tpool.tile([128, A], f32r, tag="tp")
                    for c in range(2):
                        nc.tensor.transpose(
                            out=r(tp[:, c * 128:(c + 1) * 128]),
                            in_=r(nbr_g[:, j * A + c * 128: j * A + (c + 1) * 128]),
                            identity=r(ident[:]),
                        )
                    nbrT = wpool.tile([128, A], f32r, tag="nbrT")
                    nc.scalar.copy(nbrT[:], tp[:])

                    # z = E@atom_part + nbrT'@W2 + bondT'@W3   [128 rows, 512]
                    z = zpool.tile([128, K2A], f32, tag="z")
                    nc.tensor.matmul(
                        out=z[:],
                        lhsT=r(emat[:, j * 128:(j + 1) * 128]),
                        rhs=r(ap_sb[:]),
                        start=True, stop=False,
                    )
                    for c in range(2):
                        nc.tensor.matmul(
                            out=z[:],
                            lhsT=r(nbrT[:, c * 128:(c + 1) * 128]),
                            rhs=r(wt[:, (2 + c) * K2A:(3 + c) * K2A]),
                            start=False, stop=False,
                        )
                    nc.tensor.matmul(
                        out=z[:],
                        lhsT=r(bondT[:, j * 128:(j + 1) * 128]),
                        rhs=r(wt[:, 4 * K2A:5 * K2A]),
                        start=False, stop=True,
                    )

                    # LN1 stats; rsqrt via exp(-0.5*ln(var+eps))
                    st6 = spool.tile([128, 6], f32, tag="st6")
                    nc.vector.bn_stats(st6[:], z[:])
                    st2 = spool.tile([128, 2], f32, tag="st2")
                    nc.vector.bn_aggr(st2[:], st6[:])
                    lnv = spool.tile([128, 1], f32, tag="lnv")
                    nc.scalar.activation(lnv[:], st2[:, 1:2], AF.Ln,
                                         bias=eps_t[:])
                    inv = spool.tile([128, 1], f32, tag="inv")
                    nc.scalar.activation(inv[:], lnv[:], AF.Exp, scale=-0.5)
                    ninv = spool.tile([128, 1], f32, tag="ninv")
                    nc.vector.tensor_scalar(
                        out=ninv[:], in0=inv[:], scalar1=-1.0, scalar2=None,
                        op0=mybir.AluOpType.mult,
                    )
                    # pnmi = mu*inv ; nmi = -mu*inv
                    pnmi = spool.tile([128, 1], f32, tag="pnmi")
                    nc.vector.tensor_scalar(
                        out=pnmi[:], in0=st2[:, 0:1], scalar1=inv[:],
                        scalar2=None, op0=mybir.AluOpType.mult,
                    )
                    nmi = spool.tile([128, 1], f32, tag="nmi")
                    nc.vector.tensor_scalar(
                        out=nmi[:], in0=pnmi[:], scalar1=-1.0, scalar2=None,
                        op0=mybir.AluOpType.mult,
                    )

                    # gate*core = ln(1+e^v) / (1+e^-u)
                    e_u = wpool.tile([128, A], f32, tag="e_u")
                    e_v = wpool.tile([128, A], f32, tag="e_v")
                    if general_ln1:
                        y = wpool.tile([128, K2A], f32, tag="y")
                        nc.vector.tensor_scalar(
                            out=y[:], in0=z[:], scalar1=inv[:], scalar2=nmi[:],
                            op0=mybir.AluOpType.mult, op1=mybir.AluOpType.add,
                        )
                        nc.vector.tensor_mul(out=y[:], in0=y[:], in1=g1[:])
                        nc.vector.tensor_add(out=y[:], in0=y[:], in1=b1[:])
                        nc.scalar.activation(e_u[:], y[:, :A], AF.Exp,
                                             scale=-1.0)
                        nc.scalar.activation(e_v[:], y[:, A:], AF.Exp)
                    else:
                        nc.scalar.activation(e_u[:], z[:, :A], AF.Exp,
                                             bias=pnmi[:], scale=ninv[:])
                        nc.scalar.activation(e_v[:], z[:, A:], AF.Exp,
                                             bias=nmi[:], scale=inv[:])
                    sp = wpool.tile([128, A], f32, tag="sp")
                    nc.scalar.activation(sp[:], e_v[:], AF.Ln, bias=ones_t[:])
                    denom = wpool.tile([128, A], f32, tag="denom")
                    nc.vector.tensor_scalar(
                        out=denom[:], in0=e_u[:], scalar1=1.0, scalar2=None,
                        op0=mybir.AluOpType.add,
                    )
                    rden = wpool.tile([128, A], f32, tag="rden")
                    nc.vector.reciprocal_approx_fast(out=rden[:], in_=denom[:])
                    gated = wpool.tile([128, A], f32r, tag="gated")
                    nc.vector.tensor_mul(out=gated[:], in0=sp[:], in1=rden[:])

                    # mean over m: accumulate [128 atoms, 256]
                    nc.tensor.matmul(
                        out=agg[:],
                        lhsT=r(gmat[:, j * 128:(j + 1) * 128]),
                        rhs=r(gated[:]),
                        start=(j == 0), stop=(j == RT - 1),
                    )

                # LN2 + residual
                st6b = spool.tile([128, 6], f32, tag="st6b")
                nc.vector.bn_stats(st6b[:], agg[:])
                st2b = spool.tile([128, 2], f32, tag="st2b")
                nc.vector.bn_aggr(st2b[:], st6b[:])
                lnv2 = spool.tile([128, 1], f32, tag="lnv2")
                nc.scalar.activation(lnv2[:], st2b[:, 1:2], AF.Ln,
                                     bias=eps_t[:])
                inv2 = spool.tile([128, 1], f32, tag="inv2")
                nc.scalar.activation(inv2[:], lnv2[:], AF.Exp, scale=-0.5)
                nmi2 = spool.tile([128, 1], f32, tag="nmi2")
                nc.vector.tensor_scalar(
                    out=nmi2[:], in0=st2b[:, 0:1], scalar1=inv2[:],
                    scalar2=-1.0, op0=mybir.AluOpType.mult,
                    op1=mybir.AluOpType.mult,
                )
                normed = wpool.tile([SA, A], f32, tag="normed")
                nc.vector.tensor_scalar(
                    out=normed[:], in0=agg[:], scalar1=inv2[:], scalar2=nmi2[:],
                    op0=mybir.AluOpType.mult, op1=mybir.AluOpType.add,
                )
                if general_ln2:
                    nc.vector.tensor_mul(out=normed[:], in0=normed[:], in1=g2[:])
                    nc.vector.tensor_add(out=normed[:], in0=normed[:], in1=b2[:])
                out_sb = wpool.tile([SA, A], f32, tag="out_sb")
                nc.vector.tensor_add(out=out_sb[:], in0=normed[:], in1=arows[:])
                nc.sync.dma_start(d_out[s * SA:(s + 1) * SA, :], out_sb[:])

    nc.compile()
    return nc


def _prep_inputs(atom_feats, bond_feats, fc_w, fc_b, ln1_g, ln1_b, ln2_g,
                 ln2_b, nbr_indices, general_ln1, general_ln2):
    atom_feats = np.ascontiguousarray(atom_feats, dtype=np.float32)
    pad = NPAD - N
    atom_pad = np.concatenate(
        [atom_feats, np.zeros((pad, A), np.float32)], axis=0)
    bond_pad = np.concatenate(
        [np.asarray(bond_feats, np.float32),
         np.zeros((pad, M, B), np.float32)], axis=0)
    idx_pad = np.concatenate(
        [np.asarray(nbr_indices).astype(np.int32),
         np.zeros((pad, M), np.int32)], axis=0)

    wT = np.ascontiguousarray(np.asarray(fc_w, np.float32).T)      # [640,512]
    wt_host = np.concatenate(
        [wT[c * 128:(c + 1) * 128, :] for c in range(NCHUNK)], axis=1)
    wt_host = np.ascontiguousarray(wt_host)                         # [128,2560]
    fcb_rep = np.ascontiguousarray(
        np.broadcast_to(np.asarray(fc_b, np.float32), (128, K2A)))
    emat, gmat = _expand_maps()
    ident = np.eye(128, dtype=np.float32)

    common = {"wt": wt_host, "fcb_rep": fcb_rep, "emat": emat, "gmat": gmat,
              "ident": ident, "atom_full": atom_feats}
    if general_ln1:
        common["ln1g_rep"] = np.ascontiguousarray(
            np.broadcast_to(np.asarray(ln1_g, np.float32), (128, K2A)))
        common["ln1b_rep"] = np.ascontiguousarray(
            np.broadcast_to(np.asarray(ln1_b, np.float32), (128, K2A)))
    if general_ln2:
        common["ln2g_rep"] = np.ascontiguousarray(
            np.broadcast_to(np.asarray(ln2_g, np.float32), (128, A)))
        common["ln2b_rep"] = np.ascontiguousarray(
            np.broadcast_to(np.asarray(ln2_b, np.float32), (128, A)))

    in_maps = []
    for i in range(NCORES):
        lo, hi = i * NS, (i + 1) * NS
        shard_atoms = atom_pad[lo:hi]                               # [3840,256]
        atomT = np.ascontiguousarray(shard_atoms.T)                 # [256,3840]
        atomT2 = np.ascontiguousarray(
            np.concatenate([atomT[:128], atomT[128:]], axis=1))     # [128,7680]
        bond_flat = bond_pad[lo:hi].reshape(ROWS, B)
        bondT = np.ascontiguousarray(bond_flat.T)                   # [128,46080]
        m = dict(common)
        m["atomT2"] = atomT2
        m["atom_rows"] = np.ascontiguousarray(shard_atoms)
        m["bondT"] = bondT
        m["nbr_idx"] = np.ascontiguousarray(idx_pad[lo:hi].reshape(ROWS))
        in_maps.append(m)
    return in_maps


def _run(nc, in_maps, trace=False):
    from concourse.bass_utils import run_bass_kernel_spmd
    _install_neff_cache()
    res = run_bass_kernel_spmd(nc, in_maps, list(range(NCORES)), trace=trace)
    out = np.concatenate(
        [res.results[i]["out"] for i in range(NCORES)], axis=0)[:N]
    return np.ascontiguousarray(out), res


def measure_exec_ns(nc, in_maps, iters=24):
    """Estimate device exec time by pipelining async dispatches.

    No NTFF profiling is available under this axon client, so time N
    back-to-back executions of the resident executable (inputs device-
    resident, no donation) and difference out the fixed dispatch cost.
    """
    import time

    import jax
    from jax.experimental.shard_map import shard_map
    from jax.sharding import Mesh, NamedSharding, PartitionSpec

    from concourse import bass2jax, mybir
    from concourse.bass2jax import _bass_exec_p, partition_id_tensor

    bass2jax.install_neuronx_cc_hook()
    _install_neff_cache()

    partition_name = (nc.partition_id_tensor.name
                      if nc.partition_id_tensor else None)
    in_names, out_names, out_avals, zero_outs = [], [], [], []
    for alloc in nc.m.functions[0].allocations:
        if not isinstance(alloc, mybir.MemoryLocationSet):
            continue
        name = alloc.memorylocations[0].name
        if alloc.kind == "ExternalInput":
            if name != partition_name:
                in_names.append(name)
        elif alloc.kind == "ExternalOutput":
            shape = tuple(alloc.tensor_shape)
            dtype = mybir.dt.np(alloc.dtype)
            out_names.append(name)
            out_avals.append(jax.core.ShapedArray(shape, dtype))
            zero_outs.append(np.zeros(shape, dtype))
    n_params = len(in_names)
    all_in = list(in_names) + list(out_names)
    if partition_name:
        all_in.append(partition_name)

    def _body(*args):
        operands = list(args)
        if partition_name:
            operands.append(partition_id_tensor())
        outs = _bass_exec_p.bind(
            *operands, out_avals=tuple(out_avals), in_names=tuple(all_in),
            out_names=tuple(out_names), lowering_input_output_aliases=(),
            sim_require_finite=True, sim_require_nnan=True, nc=nc)
        return tuple(outs)

    devices = jax.devices()[:NCORES]
    mesh = Mesh(np.asarray(devices), ("core",))
    nin = n_params + len(zero_outs)
    sharded = jax.jit(
        shard_map(_body, mesh=mesh, in_specs=(PartitionSpec("core"),) * nin,
                  out_specs=(PartitionSpec("core"),) * len(out_names),
                  check_rep=False),
        keep_unused=True)
    sh = NamedSharding(mesh, PartitionSpec("core"))
    concat = [np.concatenate([np.asarray(in_maps[c][nm])
                              for c in range(NCORES)], axis=0)
              for nm in in_names]
    concat += [np.zeros((NCORES * z.shape[0], *z.shape[1:]), z.dtype)
               for z in zero_outs]
    dev_in = [jax.device_put(a, sh) for a in concat]

    jax.block_until_ready(sharded(*dev_in))   # compile + warm

    def run_n(n):
        t0 = time.perf_counter()
        rs = [sharded(*dev_in) for _ in range(n)]
        jax.block_until_ready(rs)
        return time.perf_counter() - t0

    run_n(2)
    t_small = min(run_n(2) for _ in range(3))
    t_big = min(run_n(2 + iters) for _ in range(3))
    est_ns = (t_big - t_small) / iters * 1e9
    return est_ns, t_small, t_big


def kernel(atom_feats, bond_feats, fc_w, fc_b, ln1_g, ln1_b, ln2_g, ln2_b,
           nbr_indices, _trace=False, _return_res=False):
    general_ln1 = not (np.allclose(ln1_g, 1.0) and np.allclose(ln1_b, 0.0))
    general_ln2 = not (np.allclose(ln2_g, 1.0) and np.allclose(ln2_b, 0.0))
    key = (general_ln1, general_ln2)
    if key not in _CACHE:
        _CACHE[key] = _build(general_ln1, general_ln2)
    nc = _CACHE[key]
    in_maps = _prep_inputs(atom_feats, bond_feats, fc_w, fc_b, ln1_g, ln1_b,
                           ln2_g, ln2_b, nbr_indices, general_ln1, general_ln2)
    out, res = _run(nc, in_maps, trace=_trace)
    if _return_res:
        return out, res
    return out



# revision 3
# speedup vs baseline: 1.7798x; 1.7798x over previous
"""CGCNNConv forward on 8 Trainium2 NeuronCores (Bass/Tile).

Math (per atom i, neighbor slot m):
  combined = [atom[i] | atom[nbr[i,m]] | bond[i,m]]          # 640
  z        = combined @ fc_w.T + fc_b                        # 512
  z        = LN(z) * ln1_g + ln1_b
  out[i]   = atom[i] + LN( mean_m sigmoid(z[:256]) * softplus(z[256:]) ) * ln2_g + ln2_b

Sharding: atoms split across 8 cores (padded 30000 -> 30720 = 8*3840).
atom_feats is replicated to every core's HBM so the neighbor gather is a
local indirect DMA.

Device layout per core (supertile = 128 atoms = 1536 (atom,m) rows = 12
row-tiles of 128 rows):
  - atom contribution is computed once per atom (atomT stationary, W1^T
    moving) with fc_b folded in, then expanded to rows with a one-hot
    matmul (E).
  - neighbor rows are gathered [128,256] row-major and PE-transposed so
    the feature dim lands on partitions for the main matmuls.
  - bond features arrive pre-transposed from the host.
  - z accumulates in PSUM [128 rows, 512]; LN1 stats via bn_stats; the
    per-row (scale,bias) is fused into the ACT sigmoid/softplus reads.
  - mean over m via a one-hot 1/12 matmul (G) accumulating [128 atoms,256]
    across the 12 row-tiles of a supertile; LN2 + residual epilogue.
All matmuls run in float32r (full-rate fp32 mode on TRN2 PE).
"""

import os
import sys

import numpy as np

sys.path.insert(0, "/opt/trn_rl_repo")
os.environ.setdefault("NEURON_COMPILE_CACHE_URL", "/root/neff_cache")

N, M, A, B = 30000, 12, 256, 128
NCORES = 8
NS = 3840                    # atoms per core (padded)
NPAD = NS * NCORES           # 30720
SA = 128                     # atoms per supertile
NSUPER = NS // SA            # 30
RT = 12                      # row-tiles per supertile
ST_ROWS = SA * M             # 1536
ROWS = NS * M                # 46080
K2A = 2 * A                  # 512
KIN = 2 * A + B              # 640
NCHUNK = KIN // 128          # 5
LN_EPS = 1e-5

_CACHE = {}
_NEFF_CACHE_DIR = os.environ.get("NEFF_DISK_CACHE", "/root/neff_cache")
_cache_installed = False


def _install_neff_cache():
    """Cache compiled NEFFs on disk keyed by BIR hash (compile takes ~40min)."""
    global _cache_installed
    if _cache_installed:
        return
    _cache_installed = True
    import hashlib
    import shutil

    from concourse import bass2jax, bass_utils

    orig = bass_utils.compile_bir_kernel

    def cached(bir_json, tmpdir, neff_name="file.neff"):
        try:
            os.makedirs(_NEFF_CACHE_DIR, exist_ok=True)
            h = hashlib.sha256(bir_json).hexdigest()[:32]
            cpath = os.path.join(_NEFF_CACHE_DIR, h + ".neff")
            if os.path.exists(cpath):
                dst = os.path.join(tmpdir, neff_name)
                shutil.copy(cpath, dst)
                return dst
        except Exception:
            cpath = None
        out = orig(bir_json, tmpdir, neff_name)
        if cpath is not None:
            try:
                shutil.copy(out, cpath)
            except Exception:
                pass
        return out

    bass_utils.compile_bir_kernel = cached
    bass2jax.compile_bir_kernel = cached


def _expand_maps():
    """E: [atom a, (j,r)] one-hot; G: [row r, (j,a)] one-hot / 12."""
    emat = np.zeros((SA, RT * 128), dtype=np.float32)
    gmat = np.zeros((128, RT * 128), dtype=np.float32)
    for j in range(RT):
        for r in range(128):
            a = (128 * j + r) // M
            emat[a, j * 128 + r] = 1.0
            gmat[r, j * 128 + a] = 1.0 / M
    return emat, gmat


def _canonicalize_act_loads(nc):
    """Collapse the alternating exp/ln activation-table loads.

    All activation funcs this kernel uses (Copy, Ln, Exp) live together in
    act set 6 (natural_log_exp_and_others), so one load per block suffices;
    the compiler pass instead alternates exp-only/ln-only sets, inserting
    ~780 loads at ~1.3us each. The loads carry no sync info, so dropping
    the redundant ones is safe.
    """
    from concourse import mybir

    for b in nc.main_func.blocks:
        seen = False
        drop = []
        for i, inst in enumerate(b.instructions):
            if isinstance(inst, mybir.InstLoadActFuncSet):
                if seen:
                    drop.append(i)
                else:
                    inst.act_func_set_id = 6
                    seen = True
        for i in reversed(drop):
            del b.instructions[i]


def _build(general_ln1, general_ln2):
    import concourse.bass as bass
    import concourse.tile as tile
    from concourse import bacc, mybir

    f32 = mybir.dt.float32
    f32r = mybir.dt.float32r
    i32 = mybir.dt.int32
    AF = mybir.ActivationFunctionType

    nc = bacc.Bacc("TRN2", target_bir_lowering=False, debug=False,
                   num_devices=NCORES)

    d_atom_full = nc.dram_tensor("atom_full", [N, A], f32r, kind="ExternalInput")
    d_atomT = nc.dram_tensor("atomT2", [128, 2 * NS], f32r, kind="ExternalInput")
    d_atom_rows = nc.dram_tensor("atom_rows", [NS, A], f32, kind="ExternalInput")
    d_bondT = nc.dram_tensor("bondT", [B, ROWS], f32r, kind="ExternalInput")
    d_idx = nc.dram_tensor("nbr_idx", [ROWS], i32, kind="ExternalInput")
    d_wt = nc.dram_tensor("wt", [128, NCHUNK * K2A], f32r, kind="ExternalInput")
    d_fcb = nc.dram_tensor("fcb_rep", [128, K2A], f32, kind="ExternalInput")
    d_emat = nc.dram_tensor("emat", [SA, RT * 128], f32r, kind="ExternalInput")
    d_gmat = nc.dram_tensor("gmat", [128, RT * 128], f32r, kind="ExternalInput")
    d_ident = nc.dram_tensor("ident", [128, 128], f32r, kind="ExternalInput")
    if general_ln1:
        d_g1 = nc.dram_tensor("ln1g_rep", [128, K2A], f32, kind="ExternalInput")
        d_b1 = nc.dram_tensor("ln1b_rep", [128, K2A], f32, kind="ExternalInput")
    if general_ln2:
        d_g2 = nc.dram_tensor("ln2g_rep", [128, A], f32, kind="ExternalInput")
        d_b2 = nc.dram_tensor("ln2b_rep", [128, A], f32, kind="ExternalInput")
    d_out = nc.dram_tensor("out", [NS, A], f32, kind="ExternalOutput")

    r = lambda ap: ap if ap.dtype == f32r else ap.bitcast(f32r)

    with tile.TileContext(nc) as tc:
        with (
            tc.tile_pool(name="const", bufs=1) as cpool,
            tc.tile_pool(name="io", bufs=2) as iopool,
            tc.tile_pool(name="work", bufs=3) as wpool,
            tc.tile_pool(name="stat", bufs=4) as spool,
            tc.tile_pool(name="zps", bufs=2, space="PSUM") as zpool,
            tc.tile_pool(name="tps", bufs=2, space="PSUM") as tpool,
            tc.tile_pool(name="aps", bufs=2, space="PSUM") as apool,
            tc.tile_pool(name="gps", bufs=2, space="PSUM") as gpool,
        ):
            # ---- resident constants ----
            wt = cpool.tile([128, NCHUNK * K2A], f32r, tag="wt")
            nc.sync.dma_start(wt[:], d_wt[:])
            atomT = cpool.tile([128, 2 * NS], f32r, tag="atomT")
            nc.sync.dma_start(atomT[:], d_atomT[:])
            fcb = cpool.tile([128, K2A], f32, tag="fcb")
            nc.sync.dma_start(fcb[:], d_fcb[:])
            emat = cpool.tile([SA, RT * 128], f32r, tag="emat")
            nc.sync.dma_start(emat[:], d_emat[:])
            gmat = cpool.tile([128, RT * 128], f32r, tag="gmat")
            nc.sync.dma_start(gmat[:], d_gmat[:])
            ident = cpool.tile([128, 128], f32r, tag="ident")
            nc.sync.dma_start(ident[:], d_ident[:])
            eps_t = cpool.tile([128, 1], f32, tag="eps")
            nc.gpsimd.memset(eps_t[:], LN_EPS)
            ones_t = cpool.tile([128, 1], f32, tag="ones")
            nc.gpsimd.memset(ones_t[:], 1.0)
            if general_ln1:
                g1 = cpool.tile([128, K2A], f32, tag="g1")
                nc.sync.dma_start(g1[:], d_g1[:])
                b1 = cpool.tile([128, K2A], f32, tag="b1")
                nc.sync.dma_start(b1[:], d_b1[:])
            if general_ln2:
                g2 = cpool.tile([128, A], f32, tag="g2")
                nc.sync.dma_start(g2[:], d_g2[:])
                b2 = cpool.tile([128, A], f32, tag="b2")
                nc.sync.dma_start(b2[:], d_b2[:])

            for s in range(NSUPER):
                row0 = s * ST_ROWS
                # indices for this supertile: [p, j] = flat[row0 + 128j + p]
                idx = iopool.tile([128, RT], i32, tag="idx")
                nc.sync.dma_start(
                    idx[:],
                    d_idx[row0:row0 + ST_ROWS].rearrange("(j p) -> p j", p=128),
                )
                # gathered neighbor rows [p, j*256:(j+1)*256]
                nbr_g = iopool.tile([128, RT * A], f32r, tag="nbr_g")
                for j in range(RT):
                    nc.gpsimd.indirect_dma_start(
                        out=nbr_g[:, j * A:(j + 1) * A],
                        out_offset=None,
                        in_=d_atom_full[:],
                        in_offset=bass.IndirectOffsetOnAxis(
                            ap=idx[:, j:j + 1], axis=0),
                    )
                # bond^T slice [128 feat, 1536 rows]
                bondT = iopool.tile([B, ST_ROWS], f32r, tag="bondT")
                nc.sync.dma_start(bondT[:], d_bondT[:, row0:row0 + ST_ROWS])
                # residual rows
                arows = iopool.tile([SA, A], f32, tag="arows")
                nc.sync.dma_start(arows[:], d_atom_rows[s * SA:(s + 1) * SA, :])

                # atom contribution for these 128 atoms: [128 atoms, 512]
                ap_ps = apool.tile([SA, K2A], f32, tag="ap_ps")
                for c in range(2):
                    nc.tensor.matmul(
                        out=ap_ps[:],
                        lhsT=r(atomT[:, c * NS + s * SA: c * NS + (s + 1) * SA]),
                        rhs=r(wt[:, c * K2A:(c + 1) * K2A]),
                        start=(c == 0), stop=(c == 1),
                    )
                ap_sb = wpool.tile([SA, K2A], f32r, tag="ap_sb")
                nc.vector.tensor_add(out=ap_sb[:], in0=ap_ps[:], in1=fcb[:])

                agg = gpool.tile([SA, A], f32, tag="agg")

                for j in range(RT):
                    # transpose gathered neighbors -> [feat, rows]
                    tp = tpool.tile([128, A], f32r, tag="tp")
                    for c in range(2):
                        nc.tensor.transpose(
                            out=r(tp[:, c * 128:(c + 1) * 128]),
                            in_=r(nbr_g[:, j * A + c * 128: j * A + (c + 1) * 128]),
                            identity=r(ident[:]),
                        )
                    nbrT = wpool.tile([128, A], f32r, tag="nbrT")
                    nc.scalar.copy(nbrT[:], tp[:])

                    # z = E@atom_part + nbrT'@W2 + bondT'@W3   [128 rows, 512]
                    z = zpool.tile([128, K2A], f32, tag="z")
                    nc.tensor.matmul(
                        out=z[:],
                        lhsT=r(emat[:, j * 128:(j + 1) * 128]),
                        rhs=r(ap_sb[:]),
                        start=True, stop=False,
                    )
                    for c in range(2):
                        nc.tensor.matmul(
                            out=z[:],
                            lhsT=r(nbrT[:, c * 128:(c + 1) * 128]),
                            rhs=r(wt[:, (2 + c) * K2A:(3 + c) * K2A]),
                            start=False, stop=False,
                        )
                    nc.tensor.matmul(
                        out=z[:],
                        lhsT=r(bondT[:, j * 128:(j + 1) * 128]),
                        rhs=r(wt[:, 4 * K2A:5 * K2A]),
                        start=False, stop=True,
                    )

                    # LN1 stats; rsqrt via exp(-0.5*ln(var+eps))
                    st6 = spool.tile([128, 6], f32, tag="st6")
                    nc.vector.bn_stats(st6[:], z[:])
                    st2 = spool.tile([128, 2], f32, tag="st2")
                    nc.vector.bn_aggr(st2[:], st6[:])
                    lnv = spool.tile([128, 1], f32, tag="lnv")
                    nc.scalar.activation(lnv[:], st2[:, 1:2], AF.Ln,
                                         bias=eps_t[:])
                    inv = spool.tile([128, 1], f32, tag="inv")
                    nc.scalar.activation(inv[:], lnv[:], AF.Exp, scale=-0.5)
                    ninv = spool.tile([128, 1], f32, tag="ninv")
                    nc.vector.tensor_scalar(
                        out=ninv[:], in0=inv[:], scalar1=-1.0, scalar2=None,
                        op0=mybir.AluOpType.mult,
                    )
                    # pnmi = mu*inv ; nmi = -mu*inv
                    pnmi = spool.tile([128, 1], f32, tag="pnmi")
                    nc.vector.tensor_scalar(
                        out=pnmi[:], in0=st2[:, 0:1], scalar1=inv[:],
                        scalar2=None, op0=mybir.AluOpType.mult,
                    )
                    nmi = spool.tile([128, 1], f32, tag="nmi")
                    nc.vector.tensor_scalar(
                        out=nmi[:], in0=pnmi[:], scalar1=-1.0, scalar2=None,
                        op0=mybir.AluOpType.mult,
                    )

                    # gate*core = ln(1+e^v) / (1+e^-u)
                    e_u = wpool.tile([128, A], f32, tag="e_u")
                    e_v = wpool.tile([128, A], f32, tag="e_v")
                    if general_ln1:
                        y = wpool.tile([128, K2A], f32, tag="y")
                        nc.vector.tensor_scalar(
                            out=y[:], in0=z[:], scalar1=inv[:], scalar2=nmi[:],
                            op0=mybir.AluOpType.mult, op1=mybir.AluOpType.add,
                        )
                        nc.vector.tensor_mul(out=y[:], in0=y[:], in1=g1[:])
                        nc.vector.tensor_add(out=y[:], in0=y[:], in1=b1[:])
                        nc.scalar.activation(e_u[:], y[:, :A], AF.Exp,
                                             scale=-1.0)
                        nc.scalar.activation(e_v[:], y[:, A:], AF.Exp)
                    else:
                        nc.scalar.activation(e_u[:], z[:, :A], AF.Exp,
                                             bias=pnmi[:], scale=ninv[:])
                        nc.scalar.activation(e_v[:], z[:, A:], AF.Exp,
                                             bias=nmi[:], scale=inv[:])
                    sp = wpool.tile([128, A], f32, tag="sp")
                    nc.scalar.activation(sp[:], e_v[:], AF.Ln, bias=ones_t[:])
                    denom = wpool.tile([128, A], f32, tag="denom")
                    nc.vector.tensor_scalar(
                        out=denom[:], in0=e_u[:], scalar1=1.0, scalar2=None,
                        op0=mybir.AluOpType.add,
                    )
                    rden = wpool.tile([128, A], f32, tag="rden")
                    nc.vector.reciprocal_approx_fast(out=rden[:], in_=denom[:])
                    gated = wpool.tile([128, A], f32r, tag="gated")
                    nc.vector.tensor_mul(out=gated[:], in0=sp[:], in1=rden[:])

                    # mean over m: accumulate [128 atoms, 256]
                    nc.tensor.matmul(
                        out=agg[:],
                        lhsT=r(gmat[:, j * 128:(j + 1) * 128]),
                        rhs=r(gated[:]),
                        start=(j == 0), stop=(j == RT - 1),
                    )

                # LN2 + residual
                st6b = spool.tile([128, 6], f32, tag="st6b")
                nc.vector.bn_stats(st6b[:], agg[:])
                st2b = spool.tile([128, 2], f32, tag="st2b")
                nc.vector.bn_aggr(st2b[:], st6b[:])
                lnv2 = spool.tile([128, 1], f32, tag="lnv2")
                nc.scalar.activation(lnv2[:], st2b[:, 1:2], AF.Ln,
                                     bias=eps_t[:])
                inv2 = spool.tile([128, 1], f32, tag="inv2")
                nc.scalar.activation(inv2[:], lnv2[:], AF.Exp, scale=-0.5)
                nmi2 = spool.tile([128, 1], f32, tag="nmi2")
                nc.vector.tensor_scalar(
                    out=nmi2[:], in0=st2b[:, 0:1], scalar1=inv2[:],
                    scalar2=-1.0, op0=mybir.AluOpType.mult,
                    op1=mybir.AluOpType.mult,
                )
                normed = wpool.tile([SA, A], f32, tag="normed")
                nc.vector.tensor_scalar(
                    out=normed[:], in0=agg[:], scalar1=inv2[:], scalar2=nmi2[:],
                    op0=mybir.AluOpType.mult, op1=mybir.AluOpType.add,
                )
                if general_ln2:
                    nc.vector.tensor_mul(out=normed[:], in0=normed[:], in1=g2[:])
                    nc.vector.tensor_add(out=normed[:], in0=normed[:], in1=b2[:])
                out_sb = wpool.tile([SA, A], f32, tag="out_sb")
                nc.vector.tensor_add(out=out_sb[:], in0=normed[:], in1=arows[:])
                nc.sync.dma_start(d_out[s * SA:(s + 1) * SA, :], out_sb[:])

    nc.compile()
    _canonicalize_act_loads(nc)
    return nc


def _prep_inputs(atom_feats, bond_feats, fc_w, fc_b, ln1_g, ln1_b, ln2_g,
                 ln2_b, nbr_indices, general_ln1, general_ln2):
    atom_feats = np.ascontiguousarray(atom_feats, dtype=np.float32)
    pad = NPAD - N
    atom_pad = np.concatenate(
        [atom_feats, np.zeros((pad, A), np.float32)], axis=0)
    bond_pad = np.concatenate(
        [np.asarray(bond_feats, np.float32),
         np.zeros((pad, M, B), np.float32)], axis=0)
    idx_pad = np.concatenate(
        [np.asarray(nbr_indices).astype(np.int32),
         np.zeros((pad, M), np.int32)], axis=0)

    wT = np.ascontiguousarray(np.asarray(fc_w, np.float32).T)      # [640,512]
    wt_host = np.concatenate(
        [wT[c * 128:(c + 1) * 128, :] for c in range(NCHUNK)], axis=1)
    wt_host = np.ascontiguousarray(wt_host)                         # [128,2560]
    fcb_rep = np.ascontiguousarray(
        np.broadcast_to(np.asarray(fc_b, np.float32), (128, K2A)))
    emat, gmat = _expand_maps()
    ident = np.eye(128, dtype=np.float32)

    common = {"wt": wt_host, "fcb_rep": fcb_rep, "emat": emat, "gmat": gmat,
              "ident": ident, "atom_full": atom_feats}
    if general_ln1:
        common["ln1g_rep"] = np.ascontiguousarray(
            np.broadcast_to(np.asarray(ln1_g, np.float32), (128, K2A)))
        common["ln1b_rep"] = np.ascontiguousarray(
            np.broadcast_to(np.asarray(ln1_b, np.float32), (128, K2A)))
    if general_ln2:
        common["ln2g_rep"] = np.ascontiguousarray(
            np.broadcast_to(np.asarray(ln2_g, np.float32), (128, A)))
        common["ln2b_rep"] = np.ascontiguousarray(
            np.broadcast_to(np.asarray(ln2_b, np.float32), (128, A)))

    in_maps = []
    for i in range(NCORES):
        lo, hi = i * NS, (i + 1) * NS
        shard_atoms = atom_pad[lo:hi]                               # [3840,256]
        atomT = np.ascontiguousarray(shard_atoms.T)                 # [256,3840]
        atomT2 = np.ascontiguousarray(
            np.concatenate([atomT[:128], atomT[128:]], axis=1))     # [128,7680]
        bond_flat = bond_pad[lo:hi].reshape(ROWS, B)
        bondT = np.ascontiguousarray(bond_flat.T)                   # [128,46080]
        m = dict(common)
        m["atomT2"] = atomT2
        m["atom_rows"] = np.ascontiguousarray(shard_atoms)
        m["bondT"] = bondT
        m["nbr_idx"] = np.ascontiguousarray(idx_pad[lo:hi].reshape(ROWS))
        in_maps.append(m)
    return in_maps


def _run(nc, in_maps, trace=False):
    from concourse.bass_utils import run_bass_kernel_spmd
    _install_neff_cache()
    res = run_bass_kernel_spmd(nc, in_maps, list(range(NCORES)), trace=trace)
    out = np.concatenate(
        [res.results[i]["out"] for i in range(NCORES)], axis=0)[:N]
    return np.ascontiguousarray(out), res


def measure_exec_ns(nc, in_maps, iters=24):
    """Estimate device exec time by pipelining async dispatches.

    No NTFF profiling is available under this axon client, so time N
    back-to-back executions of the resident executable (inputs device-
    resident, no donation) and difference out the fixed dispatch cost.
    """
    import time

    import jax
    from jax.experimental.shard_map import shard_map
    from jax.sharding import Mesh, NamedSharding, PartitionSpec

    from concourse import bass2jax, mybir
    from concourse.bass2jax import _bass_exec_p, partition_id_tensor

    bass2jax.install_neuronx_cc_hook()
    _install_neff_cache()

    partition_name = (nc.partition_id_tensor.name
                      if nc.partition_id_tensor else None)
    in_names, out_names, out_avals, zero_outs = [], [], [], []
    for alloc in nc.m.functions[0].allocations:
        if not isinstance(alloc, mybir.MemoryLocationSet):
            continue
        name = alloc.memorylocations[0].name
        if alloc.kind == "ExternalInput":
            if name != partition_name:
                in_names.append(name)
        elif alloc.kind == "ExternalOutput":
            shape = tuple(alloc.tensor_shape)
            dtype = mybir.dt.np(alloc.dtype)
            out_names.append(name)
            out_avals.append(jax.core.ShapedArray(shape, dtype))
            zero_outs.append(np.zeros(shape, dtype))
    n_params = len(in_names)
    all_in = list(in_names) + list(out_names)
    if partition_name:
        all_in.append(partition_name)

    def _body(*args):
        operands = list(args)
        if partition_name:
            operands.append(partition_id_tensor())
        outs = _bass_exec_p.bind(
            *operands, out_avals=tuple(out_avals), in_names=tuple(all_in),
            out_names=tuple(out_names), lowering_input_output_aliases=(),
            sim_require_finite=True, sim_require_nnan=True, nc=nc)
        return tuple(outs)

    devices = jax.devices()[:NCORES]
    mesh = Mesh(np.asarray(devices), ("core",))
    nin = n_params + len(zero_outs)
    sharded = jax.jit(
        shard_map(_body, mesh=mesh, in_specs=(PartitionSpec("core"),) * nin,
                  out_specs=(PartitionSpec("core"),) * len(out_names),
                  check_rep=False),
        keep_unused=True)
    sh = NamedSharding(mesh, PartitionSpec("core"))
    concat = [np.concatenate([np.asarray(in_maps[c][nm])
                              for c in range(NCORES)], axis=0)
              for nm in in_names]
    concat += [np.zeros((NCORES * z.shape[0], *z.shape[1:]), z.dtype)
               for z in zero_outs]
    dev_in = [jax.device_put(a, sh) for a in concat]

    jax.block_until_ready(sharded(*dev_in))   # compile + warm

    def run_n(n):
        t0 = time.perf_counter()
        rs = [sharded(*dev_in) for _ in range(n)]
        jax.block_until_ready(rs)
        return time.perf_counter() - t0

    run_n(2)
    t_small = min(run_n(2) for _ in range(3))
    t_big = min(run_n(2 + iters) for _ in range(3))
    est_ns = (t_big - t_small) / iters * 1e9
    return est_ns, t_small, t_big


def kernel(atom_feats, bond_feats, fc_w, fc_b, ln1_g, ln1_b, ln2_g, ln2_b,
           nbr_indices, _trace=False, _return_res=False):
    general_ln1 = not (np.allclose(ln1_g, 1.0) and np.allclose(ln1_b, 0.0))
    general_ln2 = not (np.allclose(ln2_g, 1.0) and np.allclose(ln2_b, 0.0))
    key = (general_ln1, general_ln2)
    if key not in _CACHE:
        _CACHE[key] = _build(general_ln1, general_ln2)
    nc = _CACHE[key]
    in_maps = _prep_inputs(atom_feats, bond_feats, fc_w, fc_b, ln1_g, ln1_b,
                           ln2_g, ln2_b, nbr_indices, general_ln1, general_ln2)
    out, res = _run(nc, in_maps, trace=_trace)
    if _return_res:
        return out, res
    return out

